# revision 23
# baseline (speedup 1.0000x reference)
"""MultiHeadAttention + RoPE kernel for 8 Trainium2 NeuronCores.

Sharding: core c in 0..7 -> batch b = c//4, head-group hg = c%4 (4 heads
each).  Each core computes its 4 heads' attention for its batch and a
partial output projection y_part = out_heads @ wo[head rows]; the host
sums the 4 partials per batch and adds bo.

Per-core dataflow (all matmuls in float32r = TF32-like, 1 cyc/row):
  - host passes xT = x[b].T so the contraction dim D is on partitions
  - QT/KT computed transposed [depth, S]; bias added during PSUM evac
    (per-partition tensor_scalar), RoPE via a signed-permutation matmul
    (rot) + cos/sin elementwise combines
  - V computed natural [S, depth] with bias via a K=1 ones matmul;
    a ones column is appended -> V' [S, 65]
  - scores computed transposed: matmul(lhsT=KT tile, rhs=QT)
    -> ST [128 keys, S queries]; exp via ACT (scale=1/8 folded in)
  - PV: matmul(lhsT=V'_tile, rhs=expST) accumulates out'T [65, S];
    row 64 is the softmax denominator (from the ones column)
  - normalize with gpsimd partition-broadcast + fast reciprocal
  - O-proj: matmul(lhsT=outT[:, h, q-tile], rhs=wo[:, h, :]) with K=64
    accumulation over the 4 local heads
"""

import numpy as np

import concourse.bacc as bacc
import concourse.mybir as mybir
from concourse.tile import TileContext

try:  # persistent XLA compile cache: repeat processes skip the ~4min compile
    import jax as _jax
    _jax.config.update("jax_compilation_cache_dir", "/tmp/jax_comp_cache")
    _jax.config.update("jax_persistent_cache_min_compile_time_secs", 1.0)
except Exception:
    pass

B, S, DM, H, DH = 2, 2048, 1024, 16, 64
NCORES = 8
HL = 4                # heads per core
DHL = HL * DH         # 256
KCH = DM // 128       # 8 k-chunks of the model-dim contraction
SKT = S // 128        # 16 key tiles
NQ = S // 512         # 4 query chunks of 512 (PSUM bank limit)
QT_TILES = DHL // 128  # 2 m-tiles for the Q/K projections
QB = 1024              # phase-B q block
NQB = S // QB

F32 = mybir.dt.float32
F32R = mybir.dt.float32r
EXP = mybir.ActivationFunctionType.Exp
COPY = mybir.ActivationFunctionType.Copy
ADD = mybir.AluOpType.add
MULT = mybir.AluOpType.mult

_CACHE = {}


def _build_nc(exp_bufs=3, ld_bufs=3, st_bufs=2, pv_bufs=1,
              y_bufs=4, yps_bufs=2):
    nc = bacc.Bacc()
    xT = nc.dram_tensor("xT", [DM, S], F32R, kind="ExternalInput")
    wq = nc.dram_tensor("wq", [DM, DHL], F32R, kind="ExternalInput")
    wk = nc.dram_tensor("wk", [DM, DHL], F32R, kind="ExternalInput")
    wv = nc.dram_tensor("wv", [DM, DHL], F32R, kind="ExternalInput")
    wo = nc.dram_tensor("wo", [128, QT_TILES, DM], F32R, kind="ExternalInput")
    bq = nc.dram_tensor("bq", [128, QT_TILES], F32, kind="ExternalInput")
    bk = nc.dram_tensor("bk", [128, QT_TILES], F32, kind="ExternalInput")
    bv = nc.dram_tensor("bv", [1, DHL], F32R, kind="ExternalInput")
    cosT = nc.dram_tensor("cosT", [128, S], F32, kind="ExternalInput")
    sinT = nc.dram_tensor("sinT", [128, S], F32, kind="ExternalInput")
    prot = nc.dram_tensor("prot", [128, 128], F32R, kind="ExternalInput")
    y = nc.dram_tensor("y", [S, DM], F32, kind="ExternalOutput")

    with TileContext(nc) as tc:
        with tc.tile_pool(name="p0", bufs=1) as p0:
            qrope_r = p0.tile([128, QT_TILES, S], F32R)
            krope_r = p0.tile([128, QT_TILES, S], F32R)
            v_r = p0.tile([128, SKT, HL, DH + 1], F32R)
            prot_r = p0.tile([128, 128], F32R)
            ones_row_r = p0.tile([1, 128], F32R)
            ones_col_f = p0.tile([128, 1], F32)
            bv_r = p0.tile([1, DHL], F32R)
            nc.vector.memset(ones_col_f[:], 1.0)

            # ================= PHASE A =================
            with (
                tc.tile_pool(name="pa", bufs=1) as pa,
                tc.tile_pool(name="pa_ld", bufs=ld_bufs) as pa_ld,
                tc.tile_pool(name="pa_w", bufs=3) as pa_w,
                tc.tile_pool(name="pa_t", bufs=2) as pa_t,
                tc.tile_pool(name="ps_a", bufs=1, space="PSUM") as ps_a,
            ):
                cos_sb = pa.tile([128, S], F32)
                sin_sb = pa.tile([128, S], F32)
                nc.sync.dma_start(cos_sb[:], cosT[:, :])
                nc.sync.dma_start(sin_sb[:], sinT[:, :])
                nc.sync.dma_start(prot_r[:], prot[:, :])
                onesrow_f = pa.tile([1, 128], F32, tag="onesrow")
                nc.vector.memset(onesrow_f[:], 1.0)
                nc.vector.tensor_copy(ones_row_r[:], onesrow_f[:])
                # preload the exp ACT table while ACT is idle in phase A
                warm = pa.tile([1, 128], F32, tag="warm")
                nc.scalar.activation(warm[:], onesrow_f[:], EXP, scale=0.125)
                bq_sb = pa.tile([128, QT_TILES], F32, tag="bq")
                bk_sb = pa.tile([128, QT_TILES], F32, tag="bk")
                nc.sync.dma_start(bq_sb[:], bq[:, :])
                nc.sync.dma_start(bk_sb[:], bk[:, :])
                nc.sync.dma_start(bv_r[:], bv[:, :])

                # direct fp32r DMA loads: wq first (projection starts ASAP),
                # x per chunk, then wk/wv
                def load_w(wt):
                    w_r = pa_w.tile([128, KCH, DHL], F32R, tag="wr")
                    nc.sync.dma_start(
                        w_r[:], wt.rearrange("(k p) n -> p k n", p=128))
                    return w_r

                wq_r = load_w(wq)
                xT_r = pa.tile([128, KCH, S], F32R)
                for k in range(KCH):
                    nc.sync.dma_start(xT_r[:, k, :], xT[k * 128:(k + 1) * 128, :])
                wk_r = load_w(wk)
                wv_r = load_w(wv)

                # Q/K projections + RoPE, q-blocked
                for w_r, b_sb, dest in ((wq_r, bq_sb, qrope_r),
                                        (wk_r, bk_sb, krope_r)):
                    for mt in range(QT_TILES):
                        for qb_i in range(NQB):
                            q0 = qb_i * QB
                            ps = ps_a.tile([128, QB], F32, tag="qkps",
                                           bufs=2)
                            for nq in range(QB // 512):
                                for k in range(KCH):
                                    nc.tensor.matmul(
                                        ps[:, nq * 512:(nq + 1) * 512],
                                        w_r[:, k, mt * 128:(mt + 1) * 128],
                                        xT_r[:, k, q0 + nq * 512:q0 + (nq + 1) * 512],
                                        start=(k == 0), stop=(k == KCH - 1))
                            qb_r = pa_t.tile([128, QB], F32R, tag="qb")
                            nc.vector.tensor_scalar(
                                out=qb_r[:], in0=ps[:],
                                scalar1=b_sb[:, mt:mt + 1],
                                scalar2=None, op0=ADD)
                            ps2 = ps_a.tile([128, QB], F32, tag="rotps")
                            for nq in range(QB // 512):
                                nc.tensor.matmul(
                                    ps2[:, nq * 512:(nq + 1) * 512],
                                    prot_r[:, :],
                                    qb_r[:, nq * 512:(nq + 1) * 512],
                                    start=True, stop=True)
                            t1 = pa_t.tile([128, QB], F32, tag="t1")
                            nc.vector.scalar_tensor_tensor(
                                out=t1[:], in0=ps[:],
                                scalar=b_sb[:, mt:mt + 1],
                                in1=cos_sb[:, q0:q0 + QB], op0=ADD, op1=MULT)
                            t2 = pa_t.tile([128, QB], F32, tag="t2")
                            nc.vector.tensor_mul(t2[:], ps2[:],
                                                 sin_sb[:, q0:q0 + QB])
                            nc.vector.tensor_add(dest[:, mt, q0:q0 + QB],
                                                 t1[:], t2[:])

                # V projection (same psum scope -> can interleave)
                nc.vector.tensor_copy(
                    v_r[:, :, :, DH:DH + 1],
                    ones_col_f[:, None, None, :].broadcast_to([128, SKT, HL, 1]))
                for sk in range(SKT):
                    vps = ps_a.tile([128, DHL], F32, tag="vps", bufs=2)
                    for k in range(KCH):
                        nc.tensor.matmul(
                            vps[:], xT_r[:, k, sk * 128:(sk + 1) * 128],
                            wv_r[:, k, :],
                            start=(k == 0), stop=False)
                    nc.tensor.matmul(vps[:], ones_row_r[:], bv_r[:],
                                     start=False, stop=True)
                    nc.scalar.activation(
                        v_r[:, sk, :, 0:DH],
                        vps[:].rearrange("p (h d) -> p h d", h=HL), COPY)

            # ================= PHASE B (q-blocked) =================
            with (
                tc.tile_pool(name="pb", bufs=1) as pb,
                tc.tile_pool(name="pb_exp", bufs=exp_bufs) as pb_exp,
                tc.tile_pool(name="pb_n", bufs=2) as pb_n,
                tc.tile_pool(name="pc", bufs=1) as pc,
                tc.tile_pool(name="pc_y", bufs=y_bufs) as pc_y,
            ):
                outT_r = pb.tile([128, QT_TILES, S], F32R)
                wo_r = pc.tile([128, QT_TILES, DM], F32R)
                nc.sync.dma_start(wo_r[:], wo[:, :, :])

                ps_b_ctx = tc.tile_pool(name="ps_b", bufs=1, space="PSUM")
                ps_b = ps_b_ctx.__enter__()
                for qb_i in range(NQB):
                    q0 = qb_i * QB
                    for h in range(HL):
                        mt = h // 2
                        half = (h % 2) * DH
                        qt_h = qrope_r[half:half + DH, mt, :]
                        kt_h = krope_r[half:half + DH, mt, :]
                        pv_ps = ps_b.tile([DH + 1, QB], F32, tag="pvps",
                                          bufs=pv_bufs)
                        for sk in range(SKT):
                            st_ps = ps_b.tile([128, QB], F32, tag="stps",
                                              bufs=st_bufs)
                            for nq in range(QB // 512):
                                nc.tensor.matmul(
                                    st_ps[:, nq * 512:(nq + 1) * 512],
                                    kt_h[:, sk * 128:(sk + 1) * 128],
                                    qt_h[:, q0 + nq * 512:q0 + (nq + 1) * 512],
                                    start=True, stop=True)
                            expst = pb_exp.tile([128, QB], F32R, tag="expst")
                            nc.scalar.activation(expst[:], st_ps[:], EXP,
                                                 scale=0.125)
                            for nq in range(QB // 512):
                                nc.tensor.matmul(
                                    pv_ps[:, nq * 512:(nq + 1) * 512],
                                    v_r[:, sk, h, :],
                                    expst[:, nq * 512:(nq + 1) * 512],
                                    start=(sk == 0), stop=(sk == SKT - 1))
                        den_t = pb_n.tile([1, QB], F32, tag="dent")
                        nc.vector.tensor_copy(den_t[0:1, :],
                                              pv_ps[DH:DH + 1, :])
                        pvf = pb_n.tile([DH, QB], F32, tag="pvf")
                        nc.vector.tensor_copy(pvf[:], pv_ps[0:DH, :])
                        rec_b = pb_n.tile([DH, QB], F32, tag="recb")
                        nc.gpsimd.partition_broadcast(rec_b[:], den_t[0:1, :])
                        nc.vector.reciprocal_approx_fast(out=rec_b[:],
                                                         in_=rec_b[:])
                        nc.vector.tensor_mul(
                            outT_r[half:half + DH, mt, q0:q0 + QB],
                            pvf[:], rec_b[:])
                    # phase C for this q block (hidden under next block's B;
                    # the last block reuses the stps slots for pipelining)
                    last = qb_i == NQB - 1
                    for qt in range(q0 // 128, (q0 + QB) // 128):
                        y_ps = ps_b.tile([128, DM], F32,
                                         tag="stps" if last else "yps",
                                         bufs=st_bufs if last else 1)
                        for kc in range(QT_TILES):
                            for c2 in range(DM // 512):
                                nc.tensor.matmul(
                                    y_ps[:, c2 * 512:(c2 + 1) * 512],
                                    outT_r[:, kc, qt * 128:(qt + 1) * 128],
                                    wo_r[:, kc, c2 * 512:(c2 + 1) * 512],
                                    start=(kc == 0), stop=(kc == QT_TILES - 1))
                        y_sb = pc_y.tile([128, DM], F32, tag="ysb")
                        nc.vector.tensor_copy(y_sb[:], y_ps[:])
                        nc.sync.dma_start(y[qt * 128:(qt + 1) * 128, :], y_sb[:])
                ps_b_ctx.__exit__(None, None, None)

    nc.finalize()
    return nc



def _rope_tables():
    inv_freq = 1.0 / (10000.0 ** (np.arange(0, DH, 2, dtype=np.float32) / DH))
    ang = np.arange(S, dtype=np.float32)[:, None] * inv_freq[None, :]
    sin = np.concatenate([np.sin(ang), np.sin(ang)], axis=-1)  # [S, DH]
    cos = np.concatenate([np.cos(ang), np.cos(ang)], axis=-1)
    sinT = np.ascontiguousarray(np.vstack([sin.T, sin.T]), dtype=np.float32)
    cosT = np.ascontiguousarray(np.vstack([cos.T, cos.T]), dtype=np.float32)
    return sinT, cosT  # [128, S]


def _rot_matrix():
    half = DH // 2
    m64 = np.zeros((DH, DH), dtype=np.float32)
    for d in range(half):
        m64[d + half, d] = -1.0       # rot[d] = -q[d+half]
    for d in range(half, DH):
        m64[d - half, d] = 1.0        # rot[d] = q[d-half]
    m = np.zeros((128, 128), dtype=np.float32)
    m[0:DH, 0:DH] = m64
    m[DH:, DH:] = m64
    return m


def _make_runner(nc):
    """Build a cached jitted SPMD executor (mirrors the multi-core tail of
    concourse.bass2jax.run_bass_via_pjrt so repeat calls skip recompiles)."""
    import jax
    import numpy as _np
    from jax.sharding import Mesh, PartitionSpec
    from jax.experimental.shard_map import shard_map
    from concourse import bass2jax, mybir as _mybir

    bass2jax.install_neuronx_cc_hook()

    partition_name = (
        nc.partition_id_tensor.name if nc.partition_id_tensor else None)
    in_names, out_names, out_avals, zero_shapes = [], [], [], []
    for alloc in nc.m.functions[0].allocations:
        if not isinstance(alloc, _mybir.MemoryLocationSet):
            continue
        name = alloc.memorylocations[0].name
        if alloc.kind == "ExternalInput":
            if name != partition_name:
                in_names.append(name)
        elif alloc.kind == "ExternalOutput":
            out_names.append(name)
            shape = tuple(alloc.tensor_shape)
            dtype = _mybir.dt.np(alloc.dtype)
            out_avals.append(jax.core.ShapedArray(shape, dtype))
            zero_shapes.append((shape, dtype))
    n_params = len(in_names)
    all_names = in_names + out_names
    if partition_name is not None:
        all_names = all_names + [partition_name]

    def _body(*args):
        operands = list(args)
        if partition_name is not None:
            operands.append(bass2jax.partition_id_tensor())
        outs = bass2jax._bass_exec_p.bind(
            *operands,
            out_avals=tuple(out_avals),
            in_names=tuple(all_names),
            out_names=tuple(out_names),
            lowering_input_output_aliases=(),
            sim_require_finite=True,
            sim_require_nnan=True,
            nc=nc,
        )
        return tuple(outs)

    devices = jax.devices()[:NCORES]
    mesh = Mesh(_np.asarray(devices), ("core",))
    n_outs = len(out_names)
    sharded = jax.jit(
        shard_map(
            _body, mesh=mesh,
            in_specs=(PartitionSpec("core"),) * (n_params + n_outs),
            out_specs=(PartitionSpec("core"),) * n_outs,
            check_rep=False,
        ),
        donate_argnums=tuple(range(n_params, n_params + n_outs)),
        keep_unused=True,
    )

    def run(in_maps):
        concat_in = [
            _np.concatenate([_np.asarray(m[name]) for m in in_maps], axis=0)
            for name in in_names
        ]
        concat_zeros = [
            _np.zeros((NCORES * s[0], *s[1:]), dt) for (s, dt) in zero_shapes
        ]
        out_arrs = sharded(*concat_in, *concat_zeros)
        return [
            {
                name: _np.asarray(out_arrs[i]).reshape(
                    NCORES, *out_avals[i].shape)[c]
                for i, name in enumerate(out_names)
            }
            for c in range(NCORES)
        ]

    return run


def _get_runner():
    if "runner" not in _CACHE:
        nc = _build_nc()
        _CACHE["nc"] = nc
        _CACHE["runner"] = _make_runner(nc)
    return _CACHE["runner"]


def make_in_maps(x, wq, bq, wk, bk, wv, bv, wo, bo):
    """Build the 8 per-core input dicts from full inputs."""
    x = np.asarray(x, dtype=np.float32)
    if "tables" not in _CACHE:
        _CACHE["tables"] = _rope_tables()
        _CACHE["prot"] = _rot_matrix()
    sinT, cosT = _CACHE["tables"]
    prot = _CACHE["prot"]
    in_maps = []
    for c in range(NCORES):
        b, hg = divmod(c, HL)
        sl = slice(hg * DHL, (hg + 1) * DHL)
        in_maps.append({
            "xT": np.ascontiguousarray(x[b].T),
            "wq": np.ascontiguousarray(np.asarray(wq, np.float32)[:, sl]),
            "wk": np.ascontiguousarray(np.asarray(wk, np.float32)[:, sl]),
            "wv": np.ascontiguousarray(np.asarray(wv, np.float32)[:, sl]),
            "wo": np.ascontiguousarray(
                np.asarray(wo, np.float32)[sl, :].reshape(QT_TILES, 128, DM)
                .transpose(1, 0, 2)),
            "bq": np.ascontiguousarray(
                np.asarray(bq, np.float32)[sl].reshape(QT_TILES, 128).T),
            "bk": np.ascontiguousarray(
                np.asarray(bk, np.float32)[sl].reshape(QT_TILES, 128).T),
            "bv": np.ascontiguousarray(
                np.asarray(bv, np.float32)[sl].reshape(1, DHL)),
            "cosT": cosT,
            "sinT": sinT,
            "prot": prot,
        })
    return in_maps


def kernel(x, wq, bq, wk, bk, wv, bv, wo, bo):
    runner = _get_runner()
    in_maps = make_in_maps(x, wq, bq, wk, bk, wv, bv, wo, bo)
    results = runner(in_maps)
    bo = np.asarray(bo, dtype=np.float32)
    out = np.empty((B, S, DM), dtype=np.float32)
    for b in range(B):
        acc = results[b * HL + 0]["y"].astype(np.float32, copy=True)
        for hg in range(1, HL):
            acc += results[b * HL + hg]["y"]
        out[b] = acc + bo[None, :]
    return out
